# revision 39
# baseline (speedup 1.0000x reference)
"""8-core Trainium2 Bass kernel for nn_Attention_54778012893378.

Tensor-parallel over heads (2 heads/core), fully software-pipelined:
  per 512-query chunk sc: q/k projections (bf16 x/weights, fp32 psum), RoPE
  fused with the per-query score scaling folded into q's cos/sin tables
  (6-op drain, no staging copies: partition-offset psum*sbuf products; q
  heads drain on VectorE while k heads drain on GpSimd so both hide under
  the v-projection matmuls), then IMMEDIATELY the attention chunk qc=sc for
  both local heads interleaved (bf16 scores, exp on ScalarE narrowed to the
  unmasked column range, causal diagonal via one shared [128,128] triangular
  multiply, PV pipelined one k-tile behind scores). PSUM is two shared
  4-bank rings: mm (q/k proj + scores + denominators + wo) and ov (v proj +
  PV accumulators).

  Output rows are sharded as query blocks {128j, 1024+128j}: the exchange
  splits into two 512 KB all-to-alls (queries 0-1023 fires ~1/3 into the
  kernel, queries 1024-2047 right after the last chunk), each carrying both
  heads. Collectives + lhs gathers live on the sync queue (idle after input
  streaming) so their peer-rendezvous waits cannot block compute issue; a
  tiny warm-up collective absorbs first-collective setup. The output
  projection (bf16, two 128-row groups) overlaps the second collective.
  wo streams on the gpsimd queue, gated on sc0's compute via an overwritten
  pre-write data dependency. Host packs all inputs into [128, big-line]
  layouts and interleaves the 8 cores' row groups on gather.
"""

import numpy as np
import ml_dtypes

import concourse.bass as bass
import concourse.bacc as bacc
import concourse.tile as tile
import concourse.mybir as mybir
from concourse.bass_utils import run_bass_kernel_spmd

F32 = mybir.dt.float32
F32R = mybir.dt.float32r
BF16 = mybir.dt.bfloat16
AF = mybir.ActivationFunctionType
bf16 = ml_dtypes.bfloat16

# problem dims (hardcoded per spec)
S, D, H, HD, NC = 2048, 2048, 16, 128, 8
HL = H // NC            # local heads per core
CW = HL * HD            # per-core head-column width
QC_W = 512              # query chunk width
NQC = S // QC_W         # query chunks
NST = QC_W // 128       # k-tiles per query chunk band
NDT = D // 128          # contraction tiles over model dim
NKT = S // 128          # kpos tiles

DEBUG = False


def _rope_drain(nc, rtmp, ps, out_sl, cs, w, pfx, comb):
    """Rope from psum [te;to] without staging copies:
      out[0:64]  = te*c - to*s ; out[64:128] = te*s + to*c.
    cs = [c;s] packed [128, w]. The four psum*sbuf products (mixed operands
    may differ in base partition) run on VectorE writing base-0 tiles; the
    SBUF-only combines run on `comb` (gpsimd for k, vector for q — gpsimd
    cannot read PSUM and is ~2x slower per op, so it gets only half)."""
    A = rtmp.tile([64, w], F32, tag=pfx + "A", name=pfx + "A")
    B = rtmp.tile([64, w], F32, tag=pfx + "B", name=pfx + "B")
    C = rtmp.tile([64, w], F32, tag=pfx + "C", name=pfx + "C")
    Dt = rtmp.tile([64, w], F32, tag=pfx + "D", name=pfx + "D")
    nc.vector.tensor_mul(A, ps[0:64, :], cs[0:64, :])        # te*c
    nc.vector.tensor_mul(B, ps[64:128, :], cs[64:128, :])    # to*s
    nc.vector.tensor_mul(C, ps[0:64, :], cs[64:128, :])      # te*s
    nc.vector.tensor_mul(Dt, ps[64:128, :], cs[0:64, :])     # to*c
    comb.tensor_sub(out_sl[0:64, :], A, B)
    comb.tensor_add(out_sl[64:128, :], C, Dt)


def build_nc(causal, s=S, d=D):
    assert HL == 2, "cq/sq packing assumes 2 local heads"
    ndt, nkt, nqc, nst = NDT, NKT, NQC, NST
    hdt = ndt // 2          # dts per half-chunk

    nc = bacc.Bacc("TRN2", target_bir_lowering=False, debug=False, num_devices=NC)

    # host-packed inputs (all large contiguous lines)
    xp = nc.dram_tensor("xp", [nqc, 128, ndt * QC_W], BF16, kind="ExternalInput").ap()
    wq = nc.dram_tensor("wq", [128, ndt * CW], BF16, kind="ExternalInput").ap()
    wk = nc.dram_tensor("wk", [128, ndt * CW], BF16, kind="ExternalInput").ap()
    wv = nc.dram_tensor("wv", [128, ndt * CW], BF16, kind="ExternalInput").ap()
    wo = nc.dram_tensor("wo", [128, ndt * d], BF16, kind="ExternalInput").ap()
    cq = nc.dram_tensor("cq", [128, s], BF16, kind="ExternalInput").ap()
    sq = nc.dram_tensor("sq", [128, s], BF16, kind="ExternalInput").ap()
    cksk = nc.dram_tensor("cksk", [128, s], BF16, kind="ExternalInput").ap()
    if causal:
        # single lower-left triangular pattern P[dk,dq]=1 iff dk<=dq
        em = nc.dram_tensor("em", [128, 128], BF16, kind="ExternalInput").ap()
    else:
        em = nc.dram_tensor("em", [s, s], BF16, kind="ExternalInput").ap()
    out = nc.dram_tensor("out", [2 * 128, d], F32, kind="ExternalOutput").ap()
    if DEBUG:
        dbg_qk = nc.dram_tensor("dbg_qk", [128, 2 * HL * s], BF16, kind="ExternalOutput").ap()
        dbg_v = nc.dram_tensor("dbg_v", [128, NKT * CW], BF16, kind="ExternalOutput").ap()
        dbg_lhs = nc.dram_tensor("dbg_lhs", [128, 2 * NC * HL * 128], BF16, kind="ExternalOutput").ap()

    import contextlib
    from concourse.tile import add_dep_helper

    with tile.TileContext(nc, num_cores=NC) as tc:
        with contextlib.ExitStack() as top:
            qkv = top.enter_context(tc.tile_pool(name="qkv", bufs=1))
            qT_s = qkv.tile([128, HL, s], BF16)
            kT_s = qkv.tile([128, HL, s], BF16)
            v_s = qkv.tile([128, nkt, CW], BF16)
            dram = top.enter_context(tc.tile_pool(name="dram", bufs=1, space="DRAM"))
            # per-half exchange: block j = [128 hd-part, (2 heads x 128 q)]
            a2a_in = [dram.tile([NC, 128, HL * 128], BF16, name=f"a2ain{_g}") for _g in range(2)]
            a2a_out = [dram.tile([NC, 128, HL * 128], BF16, name=f"a2aout{_g}") for _g in range(2)]

            # tiny warm-up collective: absorbs the first-collective setup cost
            # while phase 1 runs (gpsimd; its rendezvous is immediate)
            warm_i = dram.tile([NC, 1, 64], BF16, name="warm_i")
            warm_o = dram.tile([NC, 1, 64], BF16, name="warm_o")
            wz = qkv.tile([1, NC * 64], BF16)
            nc.vector.memset(wz, 0.0)
            nc.sync.dma_start(warm_i.rearrange("a b c -> b (a c)"), wz)
            nc.gpsimd.collective_compute(
                "AllToAll",
                mybir.AluOpType.bypass,
                replica_groups=[list(range(NC))],
                ins=[warm_i.opt()],
                outs=[warm_o.opt()],
            )

            # PE pre-warm: ramp the tensor-engine clock while the first input
            # DMAs are in flight. Results are never read.
            with contextlib.ExitStack() as pw:
                warmp = pw.enter_context(tc.tile_pool(name="warmp", bufs=1))
                warmps = pw.enter_context(tc.tile_pool(name="warmps", bufs=1, space="PSUM"))
                wmt = warmp.tile([128, 512], BF16)
                nc.vector.memset(wmt, 0.0)
                wps = warmps.tile([128, 512], F32)
                for _ in range(12):
                    nc.tensor.matmul(wps, lhsT=wmt[:, 0:128], rhs=wmt, start=True, stop=True)

            # shared psum rings: mm = q/k proj + scores + denom + wo;
            # ov = v proj + PV accumulators
            mm = top.enter_context(tc.tile_pool(name="mm", bufs=2, space="PSUM"))
            ov = top.enter_context(tc.tile_pool(name="ov", bufs=4, space="PSUM"))
            rtmp = top.enter_context(tc.tile_pool(name="rtmp", bufs=1))
            ep = top.enter_context(tc.tile_pool(name="ep", bufs=2))
            accp = top.enter_context(tc.tile_pool(name="accp", bufs=2))
            att2p = top.enter_context(tc.tile_pool(name="att2p", bufs=2))
            p4 = top.enter_context(tc.tile_pool(name="p4", bufs=1))
            c2 = top.enter_context(tc.tile_pool(name="c2", bufs=1))
            ones_s = c2.tile([128, 128], BF16)
            nc.vector.memset(ones_s, 1.0)
            em_s = None
            emp = None
            if causal:
                em_s = c2.tile([128, 128], BF16)
                nc.sync.dma_start(em_s, em)
            else:
                emp = top.enter_context(tc.tile_pool(name="emp", bufs=4))
            # wo resident bf16, streamed on gpsimd in 4 chunks
            wol = top.enter_context(tc.tile_pool(name="wol", bufs=1))
            wo_sb = wol.tile([128, ndt * d], BF16)
            lhs_sb = [p4.tile([128, NC, HL * 128], BF16, name=f"lhs{_g}") for _g in range(2)]

            def attn_chunk(qc):
                """Both heads interleaved, PV one k-tile behind scores."""
                qsl = slice(qc * QC_W, (qc + 1) * QC_W)
                n_kt = nst * (qc + 1) if causal else nkt
                o_ps = [ov.tile([128, QC_W], F32, tag="ov", name=f"ops{qc}_{_h}") for _h in range(HL)]
                acc = accp.tile([128, HL * QC_W], BF16, tag="acc", name=f"acc{qc}")
                pend = None  # previous k-tile's (kt, e-tile)

                def emit_pv(p):
                    # boundary tiles: e columns < 128m are zero, so the PV
                    # moving range narrows to the live columns
                    pkt, et = p
                    pm = pkt - nst * qc if causal else -1
                    lo = pm * 128 if pm > 0 else 0
                    for h in range(HL):
                        nc.tensor.matmul(
                            o_ps[h][:, lo:],
                            lhsT=v_s[:, pkt, HD * h : HD * (h + 1)],
                            rhs=et[:, h * QC_W + lo : (h + 1) * QC_W],
                            start=(pkt == 0),
                            stop=(pkt == n_kt - 1),
                        )

                for kt in range(n_kt):
                    # boundary tiles: queries < 128m are fully masked, skip
                    # their score columns (psum there stays garbage; exp and
                    # PV never read it). Both heads share one 2-bank psum
                    # tile so exp/acc run as single wide ops.
                    km = kt - nst * qc if causal else -1
                    klo = km * 128 if km > 0 else 0
                    sp = mm.tile([128, HL * QC_W], F32, tag="mm", name=f"sps{qc}_{kt}")
                    for h in range(HL):
                        nc.tensor.matmul(
                            sp[:, h * QC_W + klo : (h + 1) * QC_W],
                            lhsT=kT_s[:, h, kt * 128 : (kt + 1) * 128],
                            rhs=qT_s[:, h, qc * QC_W + klo : (qc + 1) * QC_W],
                            start=True,
                            stop=True,
                        )
                    if pend is not None:
                        emit_pv(pend)
                    emt = None
                    if not causal:
                        emt = emp.tile([128, QC_W], BF16, tag="em", name=f"emt{qc}_{kt}")
                        nc.sync.dma_start(emt, em[kt * 128 : (kt + 1) * 128, qsl])
                    m = kt - nst * qc  # >=0: boundary tile (causal)
                    e = ep.tile([128, HL * QC_W], BF16, tag="e", name=f"e{qc}_{kt}")
                    if causal and m > 0:
                        for h in range(HL):
                            nc.vector.memset(e[:, h * QC_W : h * QC_W + m * 128], 0.0)
                            nc.scalar.activation(
                                e[:, h * QC_W + m * 128 : (h + 1) * QC_W],
                                sp[:, h * QC_W + m * 128 : (h + 1) * QC_W],
                                AF.Exp,
                            )
                    else:
                        nc.scalar.activation(e, sp, AF.Exp)
                    if causal and m >= 0:
                        for h in range(HL):
                            nc.vector.tensor_mul(
                                e[:, h * QC_W + m * 128 : h * QC_W + (m + 1) * 128],
                                e[:, h * QC_W + m * 128 : h * QC_W + (m + 1) * 128],
                                em_s,
                            )
                    elif not causal:
                        for h in range(HL):
                            nc.vector.tensor_mul(
                                e[:, h * QC_W : (h + 1) * QC_W],
                                e[:, h * QC_W : (h + 1) * QC_W],
                                emt,
                            )
                    if kt == 0:
                        nc.vector.tensor_copy(acc, e)
                    else:
                        nc.vector.tensor_add(acc, acc, e)
                    pend = (kt, e)
                emit_pv(pend)

                # denominator (pre-broadcast via ones stationary) + normalize
                att2 = att2p.tile([128, NST, HL, 128], BF16, tag="att2", name=f"att2_{qc}")
                d_ps = mm.tile([128, HL * QC_W], F32, tag="mm", name=f"dps{qc}")
                for h in range(HL):
                    nc.tensor.matmul(
                        d_ps[:, h * QC_W : (h + 1) * QC_W],
                        lhsT=ones_s,
                        rhs=acc[:, h * QC_W : (h + 1) * QC_W],
                        start=True, stop=True,
                    )
                rec = ep.tile([128, HL * QC_W], F32, tag="rb", name=f"rb{qc}", bufs=1)
                nc.vector.reciprocal_approx_fast(rec, d_ps)
                for h in range(HL):
                    nc.vector.tensor_mul(
                        att2[:, :, h, :], o_ps[h], rec[:, h * QC_W : (h + 1) * QC_W]
                    )
                # stage to the exchange buffer: block b gets both heads'
                # 128-query slice (dst core b of this half). Scalar queue:
                # cannot head-of-line block input (sync) or rope (gpsimd)
                half = qc // 2
                for jb in range(NST):
                    b = NST * (qc % 2) + jb
                    nc.scalar.dma_start(a2a_in[half][b], att2[:, jb])

            # ---------------- pipelined phase 1 + attention ----------------
            with contextlib.ExitStack() as p1:
                consts = p1.enter_context(tc.tile_pool(name="p1c", bufs=1))
                cq_s = consts.tile([128, s], BF16)    # head0 [c;s] (scaled)
                sq_s = consts.tile([128, s], BF16)    # head1 [c;s] (scaled)
                ck_s = consts.tile([128, s], BF16)    # k [c;s]
                wq_sb = consts.tile([128, ndt * CW], BF16)
                wk_sb = consts.tile([128, ndt * CW], BF16)
                wv_sb = consts.tile([128, ndt * CW], BF16)
                xch = p1.enter_context(tc.tile_pool(name="xch", bufs=5))

                hw = hdt * CW
                hx = hdt * QC_W
                xts = {}

                def emit_x_dma(sc, h):
                    t = xch.tile([128, hx], BF16, tag="xch", name=f"x{sc}_{h}")
                    nc.sync.dma_start(t, xp[sc, :, h * hx:(h + 1) * hx])
                    xts[(sc, h)] = t

                # fine-grained head-of-stream: the first two dt blocks of
                # wq/wk/x land after ~0.5 MB so matmuls start immediately
                # (subtile deps let per-dt reads proceed as ranges land)
                x00 = xch.tile([128, hx], BF16, tag="xch", name="x0_0")
                xts[(0, 0)] = x00
                nc.sync.dma_start(wq_sb[:, 0 : 2 * CW], wq[:, 0 : 2 * CW])
                nc.sync.dma_start(wk_sb[:, 0 : 2 * CW], wk[:, 0 : 2 * CW])
                nc.sync.dma_start(x00[:, 0 : 2 * QC_W], xp[0, :, 0 : 2 * QC_W])
                nc.sync.dma_start(wq_sb[:, 2 * CW : hw], wq[:, 2 * CW : hw])
                nc.sync.dma_start(wk_sb[:, 2 * CW : hw], wk[:, 2 * CW : hw])
                nc.sync.dma_start(x00[:, 2 * QC_W :], xp[0, :, 2 * QC_W : hx])
                emit_x_dma(0, 1)
                nc.sync.dma_start(wq_sb[:, hw:], wq[:, hw:])
                nc.sync.dma_start(wk_sb[:, hw:], wk[:, hw:])
                nc.sync.dma_start(cq_s, cq)
                nc.sync.dma_start(sq_s, sq)
                nc.sync.dma_start(ck_s, cksk)
                nc.sync.dma_start(wv_sb[:, 0:hw], wv[:, 0:hw])
                nc.sync.dma_start(wv_sb[:, hw:], wv[:, hw:])
                emit_x_dma(1, 0)
                emit_x_dma(1, 1)
                emit_x_dma(2, 0)

                def emit_wo_chunk(g):
                    # gated on sc=g's compute: the pre-write (overwritten by
                    # the DMA) reads sc g's first v tile, spreading the wo
                    # stream across the sc windows so it never starves x
                    wchunk = (ndt * d) // 4
                    nc.vector.tensor_copy(
                        wo_sb[:, g * wchunk : g * wchunk + CW], v_s[:, g * NST, :]
                    )
                    nc.gpsimd.dma_start(
                        wo_sb[:, g * wchunk:(g + 1) * wchunk],
                        wo[:, g * wchunk:(g + 1) * wchunk],
                    )

                for sc in range(nqc):
                    if 1 <= sc <= 3:
                        emit_wo_chunk(sc - 1)

                    scs = slice(sc * QC_W, (sc + 1) * QC_W)
                    # per-head-pair stagger: h's projections close halfway
                    # through the block so its rope (vector muls + gpsimd
                    # combines) overlaps the other head's projections and the
                    # v loop — attention never waits on rope latency
                    qcs = (cq_s, sq_s)
                    for h in range(HL):
                        q_ps = ov.tile([128, QC_W], F32, tag="ov", name=f"qps{sc}_{h}")
                        k_ps = ov.tile([128, QC_W], F32, tag="ov", name=f"kps{sc}_{h}")
                        for dt in range(ndt):
                            half, ldt = divmod(dt, hdt)
                            xsl = xts[(sc, half)][:, ldt * QC_W:(ldt + 1) * QC_W]
                            fl = dict(start=(dt == 0), stop=(dt == ndt - 1))
                            nc.tensor.matmul(
                                q_ps,
                                lhsT=wq_sb[:, dt * CW + HD * h : dt * CW + HD * (h + 1)],
                                rhs=xsl,
                                **fl,
                            )
                            nc.tensor.matmul(
                                k_ps,
                                lhsT=wk_sb[:, dt * CW + HD * h : dt * CW + HD * (h + 1)],
                                rhs=xsl,
                                **fl,
                            )
                        _rope_drain(nc, rtmp, k_ps, kT_s[:, h, scs],
                                    ck_s[:, scs], QC_W, "k", nc.gpsimd)
                        _rope_drain(nc, rtmp, q_ps, qT_s[:, h, scs],
                                    qcs[h][:, scs], QC_W, "q", nc.vector)
                    if sc == 2:
                        # a2a #1 emitted here (not at sc1) so its gpsimd
                        # peer-rendezvous wait sits after sc2's k-ropes and
                        # can only delay sc3's — ~30us of skew tolerance
                        nc.gpsimd.collective_compute(
                            "AllToAll",
                            mybir.AluOpType.bypass,
                            replica_groups=[list(range(NC))],
                            ins=[a2a_in[0].opt()],
                            outs=[a2a_out[0].opt()],
                        )
                        for j in range(NC):
                            nc.sync.dma_start(lhs_sb[0][:, j, :], a2a_out[0][j])
                    # v projection: st-outer so copies interleave with the
                    # next accumulation (ov ring shared with PV accumulators)
                    for st in range(NST):
                        v_ps = ov.tile([128, CW], F32, tag="ov", name=f"vps{sc}_{st}")
                        for dt in range(ndt):
                            half, ldt = divmod(dt, hdt)
                            nc.tensor.matmul(
                                v_ps,
                                lhsT=xts[(sc, half)][:, ldt * QC_W + st * 128 : ldt * QC_W + (st + 1) * 128],
                                rhs=wv_sb[:, dt * CW:(dt + 1) * CW],
                                start=(dt == 0),
                                stop=(dt == ndt - 1),
                            )
                        nc.vector.tensor_copy(v_s[:, sc * NST + st, :], v_ps)
                    # deep x prefetch (5-buffer ring): reuse distance
                    # keeps every emission behind the reused tile's readers
                    if sc == 0:
                        emit_x_dma(2, 1)
                        emit_x_dma(3, 0)
                    elif sc == 1:
                        emit_x_dma(3, 1)
                    if sc == 3:
                        emit_wo_chunk(3)

                    if DEBUG and sc == nqc - 1:
                        nc.sync.dma_start(dbg_qk[:, 0 : HL * s], qT_s)
                        nc.sync.dma_start(dbg_qk[:, HL * s :], kT_s)
                        nc.sync.dma_start(dbg_v, v_s)

                    attn_chunk(sc)
                    if sc == 3:
                        nc.gpsimd.collective_compute(
                            "AllToAll",
                            mybir.AluOpType.bypass,
                            replica_groups=[list(range(NC))],
                            ins=[a2a_in[1].opt()],
                            outs=[a2a_out[1].opt()],
                        )
                        for j in range(NC):
                            nc.sync.dma_start(lhs_sb[1][:, j, :], a2a_out[1][j])
                        if DEBUG:
                            w = NC * HL * 128
                            for _g in range(2):
                                nc.scalar.dma_start(
                                    dbg_lhs[:, _g * w : (_g + 1) * w], lhs_sb[_g]
                                )

            # ---------------- output projection ----------------
            with contextlib.ExitStack() as p2:
                outp = p2.enter_context(tc.tile_pool(name="outp", bufs=2))

                def wo_group(g):
                    """out rows [g*128:(g+1)*128] = lhs.T @ wo over 16 heads."""
                    o_sb = outp.tile([128, d], F32, tag="osb", name=f"osb{g}")
                    for nk in range(d // 512):
                        nsl = slice(nk * 512, (nk + 1) * 512)
                        w_ps = mm.tile([128, 512], F32, tag="mm", name=f"wps{g}_{nk}")
                        for j in range(NC):
                            for h in range(HL):
                                kt = HL * j + h
                                nc.tensor.matmul(
                                    w_ps,
                                    lhsT=lhs_sb[g][:, j, h * 128:(h + 1) * 128],
                                    rhs=wo_sb[:, kt * d + nk * 512 : kt * d + (nk + 1) * 512],
                                    start=(j == 0 and h == 0),
                                    stop=(j == NC - 1 and h == HL - 1),
                                )
                        nc.vector.tensor_copy(o_sb[:, nsl], w_ps)
                        nc.scalar.dma_start(out[g * 128:(g + 1) * 128, nsl], o_sb[:, nsl])

                wo_group(0)
                wo_group(1)

    nc.compile()
    return nc


def host_prep(inputs, s=S, d=D):
    x = np.ascontiguousarray(np.asarray(inputs["x"], dtype=np.float32)[0])
    wq = np.asarray(inputs["wq"], dtype=np.float32)
    wk = np.asarray(inputs["wk"], dtype=np.float32)
    wv = np.asarray(inputs["wv"], dtype=np.float32)
    wo = np.asarray(inputs["wo"], dtype=np.float32)
    ss = np.asarray(inputs["seq_scale"], dtype=np.float32).reshape(H)
    cos = np.asarray(inputs["freqs_cos"], dtype=np.float32)
    sin = np.asarray(inputs["freqs_sin"], dtype=np.float32)
    mask = np.asarray(inputs["mask"], dtype=np.float32)[0, 0]
    sll = np.asarray(inputs["section_log_len"], dtype=np.float32).reshape(s)

    zero = mask == 0.0
    causal = bool(
        np.array_equal(zero, np.tril(np.ones((s, s), bool)))
        and np.all(mask[~zero] <= -1e8)
    )

    if causal:
        # one shared diagonal-block pattern: P[dk, dq] = 1 iff dk <= dq
        em_in = np.ascontiguousarray(np.triu(np.ones((128, 128), np.float32)).astype(bf16))
    else:
        emT = np.exp(np.minimum(mask, 0.0)).T.astype(bf16)  # [kpos, q]
        em_in = np.ascontiguousarray(emT)

    perm = np.concatenate([np.arange(0, HD, 2), np.arange(1, HD, 2)])
    scale = sll / np.sqrt(HD)
    cksk = np.ascontiguousarray(np.concatenate([cos.T, sin.T], axis=0).astype(bf16))
    # x packed per query-chunk: xp[sc, p, dt*QC_W + j] = x[sc*QC_W+j, dt*128+p]
    xp = np.ascontiguousarray(
        x.astype(bf16).reshape(NQC, QC_W, NDT, 128).transpose(0, 3, 2, 1)
    ).reshape(NQC, 128, NDT * QC_W)
    # wo packed bf16: wo_p[p, g*d + n] = wo[g*128+p, n] (g = global head)
    wo_b = np.ascontiguousarray(
        wo.astype(bf16).reshape(NDT, 128, d).transpose(1, 0, 2)
    ).reshape(128, NDT * d)

    def pack_w(w):
        # [p, dt*CW + c] = w[dt*128+p, c]
        return np.ascontiguousarray(
            w.astype(bf16).reshape(NDT, 128, CW).transpose(1, 0, 2)
        ).reshape(128, NDT * CW)

    in_maps = []
    for i in range(NC):
        wq_s = np.concatenate(
            [wq[:, CW * i + HD * h : CW * i + HD * (h + 1)][:, perm] for h in range(HL)],
            axis=1,
        )
        wk_s = np.concatenate(
            [wk[:, CW * i + HD * h : CW * i + HD * (h + 1)][:, perm] for h in range(HL)],
            axis=1,
        )
        wv_s = wv[:, CW * i : CW * (i + 1)]
        # per-head packed [cos; sin] (scaled): cq = head 0, sq = head 1
        cqm = np.concatenate(
            [cos.T * (scale * ss[HL * i])[None, :],
             sin.T * (scale * ss[HL * i])[None, :]], axis=0
        )
        sqm = np.concatenate(
            [cos.T * (scale * ss[HL * i + 1])[None, :],
             sin.T * (scale * ss[HL * i + 1])[None, :]], axis=0
        )
        in_maps.append(
            {
                "xp": xp,
                "wq": pack_w(wq_s),
                "wk": pack_w(wk_s),
                "wv": pack_w(wv_s),
                "wo": wo_b,
                "cq": np.ascontiguousarray(cqm.astype(bf16)),
                "sq": np.ascontiguousarray(sqm.astype(bf16)),
                "cksk": cksk,
                "em": em_in,
            }
        )
    return in_maps, causal


_NC_CACHE = {}


def _get_nc(causal):
    if causal not in _NC_CACHE:
        _NC_CACHE[causal] = build_nc(causal)
    return _NC_CACHE[causal]


def kernel(**inputs) -> np.ndarray:
    in_maps, causal = host_prep(inputs)
    nc = _get_nc(causal)
    res = run_bass_kernel_spmd(nc, in_maps, core_ids=list(range(NC)))
    full = np.empty((S, D), dtype=np.float32)
    for j in range(NC):
        o = res.results[j]["out"]
        full[128 * j : 128 * (j + 1)] = o[0:128]
        full[1024 + 128 * j : 1024 + 128 * (j + 1)] = o[128:256]
    return full[None]
